# revision 14
# baseline (speedup 1.0000x reference)
"""Trainium2 Bass kernel for nn_Encoder_36421322670332.

2-layer LSTM encoder: x [1024, 512, 8] -> LSTM(8->64) -> LSTM(64->32),
returns final hidden state of layer 2 as [1024, 1, 32].

Strategy:
  - Data-parallel over batch: 8 cores x 128 samples.
  - Transposed state layout [hidden, batch] so the recurrent matmul needs no
    per-step transposes: gates.T = W_stacked @ [h1; h2; ones; x_t].
  - The two layers are merged into one iteration with a one-step offset
    (iteration k computes L1 step k and L2 step k-1); both layers' gates come
    from the same moving operand rhs = [h1; h2; ones; x_t] (105 rows).
  - Biases are folded via the constant ones-row; x_t rows are refreshed by a
    small partition-shifting SBUF->SBUF DMA each step, sourced from a
    PE-transposed staging of x.
  - Gate-grouped PSUM packing: one bank holds [i | f | o | g] blocks of
    96 rows (64 L1-units + 32 L2-units) x 128 batch, so ONE sigmoid
    activation covers i,f,o of both layers and ONE tanh covers g.
"""

import numpy as np

import concourse.bacc as bacc
import concourse.tile as tile
from concourse import mybir
from concourse.bass_utils import run_bass_kernel_spmd

# Problem constants (hardcoded per harness contract)
B_FULL = 1024
N_CORES = 8
BS = B_FULL // N_CORES  # 128 batch per core
T = 512
F = 8
H1 = 64
H2 = 32
NROW = H1 + H2  # 96 merged state rows
NRHS = NROW + 1 + F  # 105 rhs rows: h1 | h2 | ones | x_t
NITER = T + 1  # one extra iteration for the L2 tail step

F32 = mybir.dt.float32


def _np_dt(dt):
    if dt == mybir.dt.float32:
        return np.float32
    import ml_dtypes

    return ml_dtypes.bfloat16


def build_bass(DT=F32, DT_C=F32, DT_S=None, t_eff=T, replay=1):
    """Build the Bass program. DT: matmul operand dtype, DT_C: cell state
    dtype, DT_S: gate activation dtype (defaults to DT). t_eff < T builds a
    truncated-sequence variant for fast smoke tests. replay > 1 wraps the
    recurrence in a hardware loop executing it `replay` times (timing only;
    output is then meaningless beyond the first replay)."""
    global T, NITER
    T_SAVE = T
    T = t_eff
    NITER = T + 1
    if DT_S is None:
        DT_S = DT
    nc = bacc.Bacc("TRN2", target_bir_lowering=False, debug=False,
                   enable_asserts=False)

    x_d = nc.dram_tensor("x", [BS, T, F], F32, kind="ExternalInput")
    wh_d = nc.dram_tensor("wh", [NRHS, 4 * NROW], DT, kind="ExternalInput")
    y_d = nc.dram_tensor("y", [BS, H2], F32, kind="ExternalOutput")

    with tile.TileContext(nc) as tc:
        with (
            tc.tile_pool(name="persist", bufs=1) as pp,
            tc.tile_pool(name="gpsum", bufs=2, space="PSUM") as gp,
            tc.tile_pool(name="spool", bufs=2) as sp,
            tc.tile_pool(name="fcpool", bufs=2) as fcp,
            tc.tile_pool(name="igpool", bufs=2) as igp,
            tc.tile_pool(name="tcpool", bufs=2) as tcp,
        ):
            # ---- persistent tiles ----
            wh_sb = pp.tile([NRHS, 4 * NROW], DT, tag="wh")
            r0 = pp.tile([NRHS, BS], DT, tag="r0")
            r1 = pp.tile([NRHS, BS], DT, tag="r1")
            c0 = pp.tile([NROW, BS], DT_C, tag="c0")
            c1 = pp.tile([NROW, BS], DT_C, tag="c1")
            R = [r0, r1]
            C = [c0, c1]

            nc.sync.dma_start(out=wh_sb, in_=wh_d[:, :])

            # initial state: h=0, c=0; ones-row for bias folding
            nc.vector.memset(r0[0:NROW, :], 0.0)
            nc.vector.memset(r0[NROW:NROW + 1, :], 1.0)
            nc.vector.memset(r1[NROW:NROW + 1, :], 1.0)
            # L2 lanes of the first produced state must be zero (layer offset)
            nc.vector.memset(r1[H1:NROW, :], 0.0)
            nc.vector.memset(c0[:, :], 0.0)
            nc.vector.memset(c1[H1:NROW, :], 0.0)

            # ---- recurrence ----
            # x_0 into r0's x rows before the loop (DRAM-side AP transposes)
            nc.sync.dma_start(out=r0[NROW + 1:NRHS, :],
                              in_=x_d[:, 0, :].rearrange("b f -> f b"))

            import contextlib
            loop_cm = (tc.For_i(0, replay, 1) if replay > 1
                       else contextlib.nullcontext())
            with loop_cm:
                _emit_recurrence(nc, tc, x_d, wh_sb, R, C,
                                 gp, sp, fcp, igp, tcp, DT_S)

            # ---- output: h2 of final iteration, store transposed ----
            r_fin = R[NITER % 2]
            if DT != F32:
                out_sb = pp.tile([H2, BS], F32, tag="out")
                nc.vector.tensor_copy(out_sb, r_fin[H1:NROW, :])
                src = out_sb
            else:
                src = r_fin[H1:NROW, :]
            nc.sync.dma_start(
                out=y_d[:, :].rearrange("b h -> h b"), in_=src)

    nc.compile()
    T = T_SAVE
    NITER = T + 1
    return nc


def _emit_recurrence(nc, tc, x_d, wh_sb, R, C, gp, sp, fcp, igp, tcp, DT_S):
    if True:
        if True:
            for k in range(NITER):  # noqa: indentation kept for diff clarity
                m = H1 if k == 0 else NROW  # iteration 0: layer-1 rows only
                r_in, r_out = R[k % 2], R[(k + 1) % 2]
                c_in, c_out = C[k % 2], C[(k + 1) % 2]

                # refresh x rows of the *next* rhs tile (k+1's input).
                # (the tail iteration k=T reads stale x rows; its L1 output
                # is never consumed, so no zeroing is needed)
                if k + 1 < T:
                    nc.sync.dma_start(
                        out=r_out[NROW + 1:NRHS, :],
                        in_=x_d[:, k + 1, :].rearrange("b f -> f b"))

                g = gp.tile([NROW, 4 * BS], F32, tag="G")
                s = sp.tile([NROW, 4 * BS], DT_S, tag="S")
                fc = fcp.tile([NROW, BS], F32, tag="FC")
                ig = igp.tile([NROW, BS], F32, tag="IG")
                tc_t = tcp.tile([NROW, BS], DT_S, tag="TC")

                for gi in range(4):  # i, f, o, g gate blocks
                    nc.tensor.matmul(
                        g[0:m, gi * BS:(gi + 1) * BS],
                        wh_sb[:, gi * NROW:gi * NROW + m], r_in,
                        start=True, stop=True)

                # sigmoid over i|f|o, tanh over g -- one op each
                nc.scalar.activation(
                    s[0:m, 0:3 * BS], g[0:m, 0:3 * BS],
                    mybir.ActivationFunctionType.Sigmoid)
                nc.scalar.activation(
                    s[0:m, 3 * BS:4 * BS], g[0:m, 3 * BS:4 * BS],
                    mybir.ActivationFunctionType.Tanh)

                # c' = f*c + i*g ; h' = o * tanh(c')
                nc.vector.tensor_mul(
                    fc[0:m], s[0:m, BS:2 * BS], c_in[0:m])
                nc.vector.tensor_mul(
                    ig[0:m], s[0:m, 0:BS], s[0:m, 3 * BS:4 * BS])
                nc.vector.tensor_add(c_out[0:m], fc[0:m], ig[0:m])
                nc.scalar.activation(
                    tc_t[0:m], c_out[0:m], mybir.ActivationFunctionType.Tanh)
                nc.vector.tensor_mul(
                    r_out[0:m, :], s[0:m, 2 * BS:3 * BS], tc_t[0:m])


def prep_weights(Wih1, Whh1, bih1, bhh1, Wih2, Whh2, bih2, bhh2, DT=F32):
    """Host-side weight packing. Returns (wh, ident) numpy arrays.

    Gate blocks ordered [i, f, o, g]; within a block cols 0:64 are layer-1
    units, cols 64:96 layer-2 units. lhsT rows = rhs rows:
    0:64 h1 | 64:96 h2 | 96 ones(bias) | 97:105 x_t.
    PyTorch gate order in the weight matrices is i,f,g,o.
    """
    npdt = _np_dt(DT)
    b1 = (bih1 + bhh1).astype(np.float32)
    b2 = (bih2 + bhh2).astype(np.float32)
    rr1 = {"i": slice(0, 64), "f": slice(64, 128), "g": slice(128, 192),
           "o": slice(192, 256)}
    rr2 = {"i": slice(0, 32), "f": slice(32, 64), "g": slice(64, 96),
           "o": slice(96, 128)}
    order = ["i", "f", "o", "g"]

    wh = np.zeros((NRHS, 4 * NROW), np.float32)
    for gi, gn in enumerate(order):
        cs = gi * NROW
        # rows 0:64 (h1): L1 recurrent + L2 input contribution
        wh[0:H1, cs:cs + H1] = Whh1[rr1[gn], :].T
        wh[0:H1, cs + H1:cs + NROW] = Wih2[rr2[gn], :].T
        # rows 64:96 (h2): L2 recurrent
        wh[H1:NROW, cs + H1:cs + NROW] = Whh2[rr2[gn], :].T
        # bias row
        wh[NROW, cs:cs + H1] = b1[rr1[gn]]
        wh[NROW, cs + H1:cs + NROW] = b2[rr2[gn]]
        # x rows (L1 input weights)
        wh[NROW + 1:NRHS, cs:cs + H1] = Wih1[rr1[gn], :].T
    return wh.astype(npdt)


_CACHE = {}


def kernel(x, Wih1, Whh1, bih1, bhh1, Wih2, Whh2, bih2, bhh2,
           DT=F32, DT_C=None, DT_S=None, trace=False):
    if DT_C is None:
        DT_C = DT
    if DT_S is None:
        DT_S = DT
    key = (DT, DT_C, DT_S)
    if key not in _CACHE:
        _CACHE[key] = build_bass(DT, DT_C, DT_S)
    nc = _CACHE[key]

    x = np.asarray(x, np.float32)
    wh = prep_weights(
        np.asarray(Wih1, np.float32), np.asarray(Whh1, np.float32),
        np.asarray(bih1, np.float32), np.asarray(bhh1, np.float32),
        np.asarray(Wih2, np.float32), np.asarray(Whh2, np.float32),
        np.asarray(bih2, np.float32), np.asarray(bhh2, np.float32), DT)

    in_maps = []
    for ci in range(N_CORES):
        in_maps.append({
            "x": np.ascontiguousarray(x[ci * BS:(ci + 1) * BS]),
            "wh": wh,
        })
    res = run_bass_kernel_spmd(nc, in_maps, core_ids=list(range(N_CORES)),
                               trace=trace)
    y = np.concatenate([r["y"] for r in res.results], axis=0)
    out = y.reshape(B_FULL, 1, H2).astype(np.float32)
    if trace:
        out = (out, res)
    return out
